# revision 1
# baseline (speedup 1.0000x reference)
"""TRN2 Bass kernel for nn_Attention_11407433138456.

Multi-head self-attention, B=4 Q=K=2048 D=1024 H=16 DH=64, fp32 inputs.

Sharding (8 cores): data-parallel over B (4 batches x 2 cores), tensor-
parallel over heads (2 groups of 8 heads). Core c handles batch c//2,
head group c%2. Each core computes its 8 heads' attention and a partial
output projection; the host sums the two partials per batch (+ bias).

Per-core dataflow (all transposed orientation, zero on-device transposes):
  Qt/Kt [512,2048] = W^T-free form via matmul(lhsT=W chunk, rhs=xT chunk),
  f32r precision (~12-bit mantissa, 4x faster than fp32, ~2e-4 rel err).
  Per head pair (K=64 contraction, row-packed in PE at base partitions
  0/64): logits^T [128k,1024q] in PSUM -> ACT exp -> f16 E tiles -> DVE
  mask multiply -> PV matmul with lhsT=[V_h | ones] (M=66; row 64 gives
  the softmax denominator for free) accumulated over 16 key tiles ->
  reciprocal + partition_broadcast + normalize into O^T f16 -> output
  projection (f16) -> partial out [2048,1024] f32.
"""

import os
from contextlib import ExitStack

import numpy as np

import concourse.bass as bass
import concourse.mybir as mybir
import concourse.tile as tile
from concourse import bacc
from concourse.bass_utils import run_bass_kernel_spmd

dt = mybir.dt
AF = mybir.ActivationFunctionType

B, Q, KS, D, H, DH = 4, 2048, 2048, 1024, 16, 64
DG = 512  # hidden slice per core (8 heads)
NPAIR = 4  # head pairs per core
NKT = KS // 128  # 16 key tiles
QW = 1024  # q block width for attention
NQT = Q // QW  # 2 q blocks
VW = 66  # V' per-head width (64 dh + ones col + pad)

_CACHE = {}


def _build(repeat=1, variant='full'):
    nc = bacc.Bacc("TRN2", target_bir_lowering=False, debug=False, num_devices=8)

    xT_d = nc.dram_tensor("xT", [D, Q], dt.float32r, kind="ExternalInput").ap()
    wq_d = nc.dram_tensor("wq", [D, DG], dt.float32r, kind="ExternalInput").ap()
    wk_d = nc.dram_tensor("wk", [D, DG], dt.float32r, kind="ExternalInput").ap()
    wv_d = nc.dram_tensor("wv", [D, DG], dt.float32r, kind="ExternalInput").ap()
    wo_d = nc.dram_tensor("wo", [DG, D], dt.float16, kind="ExternalInput").ap()
    mT_d = nc.dram_tensor("maskT", [KS, Q], dt.float16, kind="ExternalInput").ap()
    out_d = nc.dram_tensor("out", [Q, D], dt.float32, kind="ExternalOutput").ap()

    with tile.TileContext(nc) as tc, ExitStack() as ctx:
        # ---- persistent pools ----
        qk_pool = ctx.enter_context(tc.tile_pool(name="qk", bufs=1))
        vv_pool = ctx.enter_context(tc.tile_pool(name="vv", bufs=1))
        ot_pool = ctx.enter_context(tc.tile_pool(name="ot", bufs=1))
        wo_pool = ctx.enter_context(tc.tile_pool(name="wop", bufs=1))
        psL = ctx.enter_context(tc.tile_pool(name="psL", bufs=2, space="PSUM"))
        psO = ctx.enter_context(tc.tile_pool(name="psO", bufs=2, space="PSUM"))

        # Qt/Kt: per pair, [128 dh, 2048 q] f32r (rows 0:64 even head,
        # 64:128 odd head)
        qt_sb = [qk_pool.tile([128, Q], dt.float32r, name=f"qt{p}", tag=f"qt{p}") for p in range(NPAIR)]
        kt_sb = [qk_pool.tile([128, Q], dt.float32r, name=f"kt{p}", tag=f"kt{p}") for p in range(NPAIR)]
        # V' per key tile: [128 keys, 8*66] f16, ones at col 64 of each head
        vv_sb = [vv_pool.tile([128, 8 * VW], dt.float16, name=f"vv{k}", tag=f"vv{k}") for k in range(NKT)]
        # O^T per pair: [128 dh, 2048 q] f16
        ot_sb = [ot_pool.tile([128, Q], dt.float16, name=f"ot{p}", tag=f"ot{p}") for p in range(NPAIR)]
        wo_sb = [wo_pool.tile([128, D], dt.float16, name=f"wo{c}", tag=f"wo{c}") for c in range(4)]
        for c in range(4):
            nc.sync.dma_start(out=wo_sb[c][:], in_=wo_d[c * 128 : (c + 1) * 128, :])

        for rep in range(repeat):
            # ---- phases A+B: projections, xT loaded in two q-halves ----
            # xa slots rotate: half 0 covers q/key cols 0:1024, half 1 the rest.
            with tc.tile_pool(name=f"wvp{rep}", bufs=1) as wv_pool, tc.tile_pool(
                name=f"xa{rep}", bufs=8
            ) as xa_pool, tc.tile_pool(name=f"wpr{rep}", bufs=2) as wpr_pool:
                wv_sb = []
                for c in range(8):
                    w = wv_pool.tile([128, DG], dt.float32r, name=f"wv{c}", tag=f"wv{c}")
                    nc.sync.dma_start(out=w[:], in_=wv_d[c * 128 : (c + 1) * 128, :])
                    wv_sb.append(w)
                for half in range(2):
                    q0 = half * 1024
                    xa = []
                    for c in range(8):
                        xt = xa_pool.tile([128, 1024], dt.float32r, name="xa", tag="xa")
                        nc.sync.dma_start(
                            out=xt[:], in_=xT_d[c * 128 : (c + 1) * 128, q0 : q0 + 1024]
                        )
                        xa.append(xt)
                    # V projection for this half's key tiles
                    for k in ([] if variant == "qk_only" else range(8 * half, 8 * half + 8)):
                        kc = k * 128 - q0
                        ps = psL.tile([128, 512], dt.float32, name="psB", tag="L")
                        for c in range(8):
                            nc.tensor.matmul(
                                ps[:],
                                lhsT=xa[c][:, kc : kc + 128],
                                rhs=wv_sb[c][:],
                                start=(c == 0),
                                stop=(c == 7),
                            )
                        nc.vector.memset(vv_sb[k][:], 1.0)
                        for h8 in range(8):
                            nc.vector.tensor_copy(
                                vv_sb[k][:, h8 * VW : h8 * VW + 64],
                                ps[:, h8 * 64 : h8 * 64 + 64],
                            )
                    # Q/K projections for this half's q columns
                    for p in ([] if variant == "v_only" else range(NPAIR)):
                        for w_d, dst in ((wq_d, qt_sb[p]), (wk_d, kt_sb[p])):
                            # one DMA: [1024, 128] pair-slice -> [128, 8x128]
                            wp = wpr_pool.tile([128, 1024], dt.float32r, name="wp", tag="wp")
                            nc.sync.dma_start(
                                out=wp[:].rearrange("p (c m) -> p c m", m=128),
                                in_=w_d[:, p * 128 : (p + 1) * 128].rearrange(
                                    "(c p) m -> p c m", p=128
                                ),
                            )
                            for nt in range(2):
                                ps = psL.tile([128, 512], dt.float32, name="psA", tag="L")
                                for c in range(8):
                                    nc.tensor.matmul(
                                        ps[:],
                                        lhsT=wp[:, c * 128 : (c + 1) * 128],
                                        rhs=xa[c][:, nt * 512 : (nt + 1) * 512],
                                        start=(c == 0),
                                        stop=(c == 7),
                                    )
                                nc.scalar.activation(
                                    dst[:, q0 + nt * 512 : q0 + (nt + 1) * 512],
                                    ps[:],
                                    AF.Copy,
                                )

            # ---- phase C: attention + output projection ----
            if variant in ("proj_only", "qk_only", "v_only"):
                continue
            with tc.tile_pool(name=f"mask{rep}", bufs=17) as mask_pool, tc.tile_pool(
                name=f"et{rep}", bufs=3
            ) as et_pool, tc.tile_pool(name=f"ep{rep}", bufs=2) as ep_pool, tc.tile_pool(
                name=f"osb{rep}", bufs=2
            ) as osb_pool:
                for qt in range(NQT):
                    mtiles = []
                    for k in range(NKT):
                        mt = mask_pool.tile([128, QW], dt.float16, name="mt", tag="mt")
                        nc.sync.dma_start(
                            out=mt[:], in_=mT_d[k * 128 : (k + 1) * 128, qt * QW : (qt + 1) * QW]
                        )
                        mtiles.append(mt)
                    for p in range(NPAIR):
                        po = [
                            psO.tile([128, QW], dt.float32, name=f"psO{h}", tag="O")
                            for h in range(2)
                        ]
                        for k in range(NKT):
                            pl = [
                                psL.tile([128, QW], dt.float32, name=f"psL{h}", tag="L")
                                for h in range(2)
                            ]
                            for h in range(2):
                                b0 = 0 if variant == "nob64" else h * 64
                                for hf in range(QW // 512):
                                    nc.tensor.matmul(
                                        pl[h][:, hf * 512 : (hf + 1) * 512],
                                        lhsT=kt_sb[p][b0 : b0 + 64, k * 128 : (k + 1) * 128],
                                        rhs=qt_sb[p][
                                            b0 : b0 + 64,
                                            qt * QW + hf * 512 : qt * QW + (hf + 1) * 512,
                                        ],
                                        start=True,
                                        stop=True,
                                    )
                            for h in range(2):
                                et = et_pool.tile([128, QW], dt.float16, name="et", tag="et")
                                nc.scalar.activation(
                                    et[:],
                                    pl[h][:],
                                    AF.Copy if variant == "no_exp" else AF.Exp,
                                )
                                if variant not in ("no_mask", "no_epmask"):
                                    nc.vector.tensor_mul(et[:], et[:], mtiles[k][:])
                                vcol = (2 * p + h) * VW
                                mw = 64 if variant == "pv64" else VW
                                for hf in range(QW // 512):
                                    nc.tensor.matmul(
                                        po[h][0:mw, hf * 512 : (hf + 1) * 512],
                                        lhsT=vv_sb[k][:, vcol : vcol + mw],
                                        rhs=et[:, hf * 512 : (hf + 1) * 512],
                                        start=(k == 0),
                                        stop=(k == NKT - 1),
                                    )
                        for h in range(2):
                            if variant in ("no_ep", "no_epmask"):
                                nc.vector.tensor_copy(
                                    ot_sb[p][h * 64 : (h + 1) * 64, qt * QW : (qt + 1) * QW],
                                    po[h][0:64, :],
                                )
                                continue
                            rec = ep_pool.tile([1, QW], dt.float32, name="rec", tag="rec")
                            nc.vector.reciprocal(rec[:], po[h][64:65, :])
                            bc = ep_pool.tile([64, QW], dt.float32, name="bc", tag="bc")
                            if variant == "no_bcast":
                                nc.vector.tensor_copy(
                                    ot_sb[p][h * 64 : (h + 1) * 64, qt * QW : (qt + 1) * QW],
                                    po[h][0:64, :],
                                )
                                continue
                            nc.gpsimd.partition_broadcast(bc[:], rec[:])
                            nc.vector.tensor_mul(
                                ot_sb[p][h * 64 : (h + 1) * 64, qt * QW : (qt + 1) * QW],
                                po[h][0:64, :],
                                bc[:],
                            )
                    # output projection for this q block
                    for qc in range(QW // 128):
                        osb = osb_pool.tile([128, D], dt.float32, name="osb", tag="osb")
                        q0 = qt * QW + qc * 128
                        for ncol in range(2):
                            pf = psL.tile([128, 512], dt.float32, name="psF", tag="L")
                            for p in range(NPAIR):
                                nc.tensor.matmul(
                                    pf[:],
                                    lhsT=ot_sb[p][:, q0 : q0 + 128],
                                    rhs=wo_sb[p][:, ncol * 512 : (ncol + 1) * 512],
                                    start=(p == 0),
                                    stop=(p == NPAIR - 1),
                                )
                            nc.vector.tensor_copy(
                                osb[:, ncol * 512 : (ncol + 1) * 512], pf[:]
                            )
                        nc.sync.dma_start(out=out_d[q0 : q0 + 128, :], in_=osb[:])

    nc.compile()
    return nc


def _get_nc():
    if "nc" not in _CACHE:
        _CACHE["nc"] = _build()
    return _CACHE["nc"]


def kernel(x, mask, Wq, Wk, Wv, Wo, bo):
    x = np.asarray(x, dtype=np.float32)
    mask_f16 = np.asarray(mask).astype(np.float16)
    Wq = np.asarray(Wq, dtype=np.float32)
    Wk = np.asarray(Wk, dtype=np.float32)
    Wv = np.asarray(Wv, dtype=np.float32)
    Wo = np.asarray(Wo, dtype=np.float32)
    bo = np.asarray(bo, dtype=np.float32)

    scale = np.float32(DH**-0.5)
    nc = _get_nc()

    in_maps = []
    for c in range(8):
        b, g = c // 2, c % 2
        gs = slice(g * DG, (g + 1) * DG)
        in_maps.append(
            {
                "xT": np.ascontiguousarray(x[b].T),
                "wq": np.ascontiguousarray(Wq[:, gs]) * scale,
                "wk": np.ascontiguousarray(Wk[:, gs]),
                "wv": np.ascontiguousarray(Wv[:, gs]),
                "wo": np.ascontiguousarray(Wo[gs, :]).astype(np.float16),
                "maskT": np.ascontiguousarray(mask_f16[b].T),
            }
        )

    res = run_bass_kernel_spmd(nc, in_maps, list(range(8))).results

    out = np.empty((B, Q, D), dtype=np.float32)
    for b in range(B):
        out[b] = res[2 * b]["out"] + res[2 * b + 1]["out"]
    out += bo
    return out

